# revision 67
# baseline (speedup 1.0000x reference)
"""Causal varlen self-attention (packed equal-length sequences) on 8 trn2 cores.

Sharding: 4 sequences x 2 head-groups. Core c handles sequence b = c//2 and
heads hh*8..hh*8+8 where hh = c%2. Each core computes the QKV projection for
its sequence restricted to its heads, rotary+RMSNorm, causal attention over
its 8 heads, and a partial output projection over its 512 features. The pair
of cores for a sequence ReduceScatter their partial y (each ends with half
the reduced rows); the host stitches the halves.

All matmuls run in bf16 (rel err ~4e-3, tolerance 2e-2). The host ships x
pre-transposed and weights pre-converted to bf16, so the device does no
f32 casts for weights and no x transpose. Attention computes only the
at-or-below-diagonal 128-col blocks (QK, exp, PV all sub-ranged). Softmax
uses exp without max subtraction (RMS-normed q,k bound |s| <= 8) in a
transposed scores layout [kpos, q]. Denominators come from a ones block
appended to V; normalization divides the small per-head attention output
using reciprocal_approx_fast. RMS mean-squares are computed from pre-rotary
values (rotation preserves norms), in parallel with the rotation itself.
"""
import numpy as np

N_EMBD = 1024
N_HEAD = 16
HD = 64
S = 1024
B = 4
N = B * S
NCORES = 8
HPC = 8             # heads per core
NHC = HPC // 2      # head-pair chunks per core
NB = S // 128       # row blocks per sequence
ND = N_EMBD // 128  # contraction chunks
JW = 3 * HPC * HD   # qkv feature width per core (1536)
NEG = -30000.0
RMS_EPS = 1.1920929e-07

_cached = {}
DEBUG = False


def _build():
    import concourse.bacc as bacc
    import concourse.mybir as mybir
    import concourse.tile as tile
    import concourse.bass as bass
    from concourse.masks import make_identity

    F32 = mybir.dt.float32
    BF16 = mybir.dt.bfloat16
    AF = mybir.ActivationFunctionType

    nc = bacc.Bacc('TRN2', target_bir_lowering=False, debug=False,
                   num_devices=NCORES)
    xt = nc.dram_tensor('xt', [N_EMBD, S], BF16, kind='ExternalInput').ap()
    # W repacked host-side as 3 contiguous [n_embd, 512] planes (q, k, v)
    wqkvT = nc.dram_tensor('wqkvT', [3 * N_EMBD, JW // 3], BF16, kind='ExternalInput').ap()
    woT = nc.dram_tensor('woT', [HPC * HD, N_EMBD], BF16, kind='ExternalInput').ap()
    # csg = [cos|sin] per position (host-gathered), one packed DMA
    csg = nc.dram_tensor('csg', [S, HD], BF16, kind='ExternalInput').ap()
    ypart = nc.dram_tensor('ypart', [S, N_EMBD], BF16, kind='ExternalOutput').ap()
    if DEBUG:
        dbg_q = nc.dram_tensor('dbg_q', [128, NHC * S], BF16, kind='ExternalOutput').ap()
        dbg_k = nc.dram_tensor('dbg_k', [128, NHC * S], BF16, kind='ExternalOutput').ap()
        dbg_att = nc.dram_tensor('dbg_att', [128, NHC * S], BF16, kind='ExternalOutput').ap()
        dbg_v = nc.dram_tensor('dbg_v', [128, NB * HPC * 128], BF16, kind='ExternalOutput').ap()
        dbg_ys = nc.dram_tensor('dbg_ys', [S, N_EMBD], BF16, kind='ExternalOutput').ap()

    def bcast_mid(t, n, width):
        # view [128, width] tile as [128, n, width] broadcasting over middle dim
        return bass.AP(tensor=t.tensor, offset=t.offset,
                       ap=[t.ap[0], [0, n], t.ap[-1]])

    def bcast_last(t, width):
        # view [128, n] tile as [128, n, width] broadcasting over last dim
        return bass.AP(tensor=t.tensor, offset=t.offset,
                       ap=[t.ap[0], t.ap[1], [0, width]])

    def view3(t, off, n, w, stride=None):
        # view [128, C] contiguous tile as [128, n, w] starting at column off
        return bass.AP(tensor=t.tensor, offset=t.offset + off,
                       ap=[t.ap[0], [w if stride is None else stride, n], [1, w]])

    with tile.TileContext(nc) as tc:
        import contextlib
        ctx = contextlib.ExitStack()
        with ctx:
            const = ctx.enter_context(tc.tile_pool(name='const', bufs=1))
            persist = ctx.enter_context(tc.tile_pool(name='persist', bufs=1))

            identb = const.tile([128, 128], BF16)
            make_identity(nc, identb)
            epst = const.tile([128, 1], F32)
            nc.vector.memset(epst, RMS_EPS)

            # persistent attention operands, bf16
            qT = persist.tile([128, NHC, S], BF16, name='qT')
            kT = persist.tile([128, NHC, S], BF16, name='kT')
            attT = persist.tile([128, NHC, S], BF16, name='attT')
            # v per head padded to 128 cols: cols 0:64 = ones, 64:128 = v, so
            # the PV matmul produces denominator rows at partitions 0:64
            # (reciprocal_approx_fast needs an offset-0 view).
            vt = [persist.tile([128, HPC, 128], BF16, name=f'vt{i}') for i in range(NB)]

            # csn[p, nb, 0:32] = cos, [32:64] = sin for position nb*128+p
            csn = const.tile([128, NB, HD], BF16, name='csn')
            nc.sync.dma_start(
                out=csn,
                in_=bass.AP(tensor=csg.tensor, offset=csg.offset,
                            ap=[[HD, 128], [128 * HD, NB], [1, HD]]))
            for nb in range(NB):
                nc.gpsimd.memset(vt[nb][:, :, 0:HD], 1.0)

            # ---- phase 1: QKV projection + rotary/RMS + head transposes ----
            with tc.tile_pool(name='xwp', bufs=1) as xwp, \
                 tc.tile_pool(name='work', bufs=2) as work, \
                 tc.tile_pool(name='psq', bufs=2, space='PSUM') as psq, \
                 tc.tile_pool(name='pst', bufs=2, space='PSUM') as pst:
                # big contiguous input DMAs first, in matmul consumption order;
                # tiny cos/sin gathers after (needed only once nb0's QKV ends)
                xT = [xwp.tile([128, S], BF16, name=f'xT{d}') for d in range(ND)]
                wq = [xwp.tile([128, JW], BF16, name=f'wq{d}') for d in range(ND)]
                for d in range(ND):
                    nc.sync.dma_start(out=xT[d], in_=xt[d * 128:(d + 1) * 128])
                    nc.sync.dma_start(
                        out=wq[d][:, 0:512],
                        in_=wqkvT[d * 128:(d + 1) * 128, :])
                for jc in range(1, 3):
                    for d in range(ND):
                        nc.sync.dma_start(
                            out=wq[d][:, jc * 512:(jc + 1) * 512],
                            in_=wqkvT[jc * N_EMBD + d * 128:jc * N_EMBD + (d + 1) * 128, :])

                for nb in range(NB):
                    pq = psq.tile([128, 3, 512], F32, tag='pq')
                    for d in range(ND):
                        for jc in range(3):
                            nc.tensor.matmul(
                                pq[:, jc, :],
                                xT[d][:, nb * 128:(nb + 1) * 128],
                                wq[d][:, jc * 512:(jc + 1) * 512],
                                start=(d == 0), stop=(d == ND - 1))

                    # qk to bf16 sbuf; mean-square from pre-rotary values
                    # (rotation preserves norms)
                    qksb = work.tile([128, 16, 64], BF16, tag='qksb')
                    nc.scalar.copy(out=qksb, in_=view3(pq, 0, 16, 64))
                    sq = work.tile([128, 16, 64], BF16, tag='sq')
                    nc.scalar.activation(out=sq, in_=qksb, func=AF.Square)
                    ms = work.tile([128, 16], F32, tag='ms')
                    nc.vector.reduce_sum(out=ms, in_=sq, axis=mybir.AxisListType.X)
                    nc.scalar.activation(out=ms, in_=ms, func=AF.Sqrt,
                                         bias=epst, scale=1.0 / HD)
                    msb = work.tile([128, 16], BF16, tag='msb')
                    with nc.allow_low_precision(reason="bf16 rms normalizer"):
                        nc.vector.reciprocal(out=msb, in_=ms)

                    x1 = qksb[:, :, 0:32]
                    x2 = qksb[:, :, 32:64]
                    cb = bcast_mid(csn[:, nb, 0:32], 16, 32)
                    sb = bcast_mid(csn[:, nb, 32:64], 16, 32)
                    ta = work.tile([128, 16, 32], BF16, tag='ta')
                    tb = work.tile([128, 16, 32], BF16, tag='tb')
                    rot = work.tile([128, 16, 64], BF16, tag='rot')
                    nc.vector.tensor_mul(ta, x1, cb)
                    nc.vector.tensor_mul(tb, x2, sb)
                    nc.vector.tensor_add(rot[:, :, 0:32], ta, tb)
                    nc.vector.tensor_mul(ta, x2, cb)
                    nc.vector.tensor_mul(tb, x1, sb)
                    nc.vector.tensor_tensor(out=rot[:, :, 32:64], in0=ta, in1=tb,
                                            op=mybir.AluOpType.subtract)
                    qkb = work.tile([128, 16, 64], BF16, tag='qkb')
                    nc.vector.tensor_mul(qkb, rot, bcast_last(msb, 64))

                    # v: psum f32 -> bf16 sbuf (second half of the padded tile)
                    nc.scalar.copy(out=vt[nb][:, :, HD:128], in_=view3(pq, 1024, 8, 64))

                    # transpose q,k head pairs: [pos, 2hd] -> [2hd, pos]
                    tp = pst.tile([128, 8, 128], BF16, tag='tp')
                    for g in range(8):
                        nc.tensor.transpose(
                            tp[:, g, :],
                            qkb[:, 2 * g:2 * g + 2, :].rearrange("p a b -> p (a b)"),
                            identb)
                    nc.vector.tensor_copy(qT[:, :, nb * 128:(nb + 1) * 128], tp[:, 0:4, :])
                    nc.scalar.copy(out=kT[:, :, nb * 128:(nb + 1) * 128], in_=tp[:, 4:8, :])

            # ---- phase 2: attention interleaved with output projection ----
            with tc.tile_pool(name='estp', bufs=6) as estp, \
                 tc.tile_pool(name='attw', bufs=3) as attw, \
                 tc.tile_pool(name='wop', bufs=1) as wop, \
                 tc.tile_pool(name='ywork', bufs=4) as yw, \
                 tc.tile_pool(name='pssc', bufs=2, space='PSUM') as pssc, \
                 tc.tile_pool(name='pspv', bufs=1, space='PSUM') as pspv, \
                 tc.tile_pool(name='psy', bufs=2, space='PSUM') as psy:
                wo = []
                for f in range(NHC):
                    wof = wop.tile([128, N_EMBD], BF16, name=f'wo{f}')
                    nc.sync.dma_start(out=wof, in_=woT[f * 128:(f + 1) * 128])
                    wo.append(wof)

                for qg in range(2):
                    for hc in range(NHC):
                        nkc = 4 + qg * 4
                        pvt = pspv.tile([128, 2, 512], F32, tag='pv')
                        for kc in range(nkc):
                            vs = max(0, kc - qg * 4) * 128
                            diag = kc >= qg * 4
                            sct = pssc.tile([128, 2, 512], F32, tag='sc')
                            for h2 in range(2):
                                nc.tensor.matmul(
                                    sct[:, h2, vs:],
                                    kT[h2 * HD:(h2 + 1) * HD, hc, kc * 128:(kc + 1) * 128],
                                    qT[h2 * HD:(h2 + 1) * HD, hc,
                                       qg * 512 + vs:(qg + 1) * 512],
                                    start=True, stop=True,
                                    tile_position=(h2 * HD, 0))
                            est = estp.tile([128, 2, 512], BF16, tag='est')
                            # attention scale D^-0.5 folded into the exp
                            nc.scalar.activation(out=est[:, :, vs:], in_=sct[:, :, vs:],
                                                 func=AF.Exp, scale=HD ** -0.5)
                            if diag:
                                # zero above-diagonal exp(s) entries on gpsimd
                                nc.gpsimd.affine_select(
                                    out=est[:, :, vs:vs + 128],
                                    in_=est[:, :, vs:vs + 128],
                                    compare_op=mybir.AluOpType.is_ge,
                                    fill=0.0, base=0, pattern=[[0, 2], [1, 128]],
                                    channel_multiplier=-1)
                            for h2 in range(2):
                                nc.tensor.matmul(
                                    pvt[:, h2, vs:], vt[kc][:, hc * 2 + h2],
                                    est[:, h2, vs:],
                                    start=(kc == 0), stop=(kc == nkc - 1),
                                    skip_group_check=True)
                        den = attw.tile([HD, 2, 512], F32, tag='den')
                        nc.vector.reciprocal_approx_fast(den, pvt[0:HD, :, :])
                        for h2 in range(2):
                            nc.vector.tensor_mul(
                                attT[h2 * HD:(h2 + 1) * HD, hc, qg * 512:(qg + 1) * 512],
                                pvt[HD:128, h2, :], den[:, h2, :])

                    # project this half's rows while the other half's attention
                    # runs; ReduceScatter in 256-row chunks for overlap
                    for qt in range(qg * 4, qg * 4 + 4):
                        for og in range(2):
                            py = psy.tile([128, 512], F32, tag='py')
                            for f in range(NHC):
                                nc.tensor.matmul(
                                    py,
                                    attT[:, f, qt * 128:(qt + 1) * 128],
                                    wo[f][:, og * 512:(og + 1) * 512],
                                    start=(f == 0), stop=(f == NHC - 1))
                            ys = yw.tile([128, 512], BF16, tag='ys')
                            if og == 0:
                                nc.vector.tensor_copy(ys, py)
                            else:
                                nc.scalar.copy(out=ys, in_=py)
                            nc.sync.dma_start(
                                out=ypart[qt * 128:(qt + 1) * 128,
                                          og * 512:(og + 1) * 512],
                                in_=ys)
                    if DEBUG and qg == 1:
                        nc.sync.dma_start(out=dbg_q, in_=qT.rearrange("p a b -> p (a b)"))
                        nc.sync.dma_start(out=dbg_k, in_=kT.rearrange("p a b -> p (a b)"))
                        nc.sync.dma_start(out=dbg_att, in_=attT.rearrange("p a b -> p (a b)"))
                        for nb in range(NB):
                            nc.sync.dma_start(
                                out=dbg_v[:, nb * 1024:(nb + 1) * 1024],
                                in_=vt[nb].rearrange("p a b -> p (a b)"))

    nc.compile()
    return nc


def _get_nc():
    if 'nc' not in _cached:
        _cached['nc'] = _build()
    return _cached['nc']


def kernel(x, Wqkv, Wo, cos_cache, sin_cache, cu_seqlens, position_ids,
           max_seqlen, **_ignored):
    import ml_dtypes
    from concourse.bass_utils import run_bass_kernel_spmd

    BF = ml_dtypes.bfloat16
    x = np.asarray(x)
    Wqkv = np.asarray(Wqkv)
    Wo = np.asarray(Wo)
    cos_cache = np.asarray(cos_cache, dtype=np.float32)
    sin_cache = np.asarray(sin_cache, dtype=np.float32)
    position_ids = np.asarray(position_ids)

    nc = _get_nc()
    in_maps = []
    for c in range(NCORES):
        b, hh = c // 2, c % 2
        rows = slice(b * S, (b + 1) * S)
        qsl = slice(hh * HPC * HD, (hh + 1) * HPC * HD)
        ksl = slice(N_EMBD + hh * HPC * HD, N_EMBD + (hh + 1) * HPC * HD)
        vsl = slice(2 * N_EMBD + hh * HPC * HD, 2 * N_EMBD + (hh + 1) * HPC * HD)
        # three planes [n_embd, 512] stacked: q, k, v
        wqkvT_c = np.concatenate(
            [Wqkv[qsl].T, Wqkv[ksl].T, Wqkv[vsl].T], axis=0)
        woT_c = Wo[:, qsl].T
        pos = position_ids[rows]
        in_maps.append({
            'xt': np.ascontiguousarray(x[rows].T.astype(BF)),
            'wqkvT': np.ascontiguousarray(wqkvT_c.astype(BF)),
            'woT': np.ascontiguousarray(woT_c.astype(BF)),
            'csg': np.ascontiguousarray(
                np.concatenate([cos_cache[pos], sin_cache[pos]], 1).astype(BF)),
        })

    r = run_bass_kernel_spmd(nc, in_maps, list(range(NCORES)))
    out = np.empty((N, N_EMBD), dtype=np.float32)
    for b in range(B):
        # unshard the PartialSum-sharded y: add the two head-group partials
        out[b * S:(b + 1) * S] = (
            r.results[2 * b]['ypart'].astype(np.float32)
            + r.results[2 * b + 1]['ypart'].astype(np.float32))
    _cached['last_results'] = r
    return out


# revision 70
# speedup vs baseline: 1.0501x; 1.0501x over previous
"""Causal varlen self-attention (packed equal-length sequences) on 8 trn2 cores.

Sharding: 4 sequences x 2 head-groups. Core c handles sequence b = c//2 and
heads hh*8..hh*8+8 where hh = c%2. Each core computes the QKV projection for
its sequence restricted to its heads, rotary+RMSNorm, causal attention over
its 8 heads, and a partial output projection over its 512 features. The pair
of cores for a sequence ReduceScatter their partial y (each ends with half
the reduced rows); the host stitches the halves.

All matmuls run in bf16 (rel err ~4e-3, tolerance 2e-2). The host ships x
pre-transposed and weights pre-converted to bf16, so the device does no
f32 casts for weights and no x transpose. Attention computes only the
at-or-below-diagonal 128-col blocks (QK, exp, PV all sub-ranged). Softmax
uses exp without max subtraction (RMS-normed q,k bound |s| <= 8) in a
transposed scores layout [kpos, q]. Denominators come from a ones block
appended to V; normalization divides the small per-head attention output
using reciprocal_approx_fast. RMS mean-squares are computed from pre-rotary
values (rotation preserves norms), in parallel with the rotation itself.
"""
import numpy as np

N_EMBD = 1024
N_HEAD = 16
HD = 64
S = 1024
B = 4
N = B * S
NCORES = 8
HPC = 8             # heads per core
NHC = HPC // 2      # head-pair chunks per core
NB = S // 128       # row blocks per sequence
ND = N_EMBD // 128  # contraction chunks
JW = 3 * HPC * HD   # qkv feature width per core (1536)
NEG = -30000.0
RMS_EPS = 1.1920929e-07

_cached = {}
DEBUG = False


def _build():
    import concourse.bacc as bacc
    import concourse.mybir as mybir
    import concourse.tile as tile
    import concourse.bass as bass
    from concourse.masks import make_identity

    F32 = mybir.dt.float32
    BF16 = mybir.dt.bfloat16
    AF = mybir.ActivationFunctionType

    nc = bacc.Bacc('TRN2', target_bir_lowering=False, debug=False,
                   num_devices=NCORES)
    xt = nc.dram_tensor('xt', [N_EMBD, S], BF16, kind='ExternalInput').ap()
    wqkvT = nc.dram_tensor('wqkvT', [N_EMBD, JW], BF16, kind='ExternalInput').ap()
    woT = nc.dram_tensor('woT', [HPC * HD, N_EMBD], BF16, kind='ExternalInput').ap()
    # csg = [cos|sin] per position (host-gathered), one packed DMA
    csg = nc.dram_tensor('csg', [S, HD], BF16, kind='ExternalInput').ap()
    ypart = nc.dram_tensor('ypart', [S, N_EMBD], BF16, kind='ExternalOutput').ap()
    if DEBUG:
        dbg_q = nc.dram_tensor('dbg_q', [128, NHC * S], BF16, kind='ExternalOutput').ap()
        dbg_k = nc.dram_tensor('dbg_k', [128, NHC * S], BF16, kind='ExternalOutput').ap()
        dbg_att = nc.dram_tensor('dbg_att', [128, NHC * S], BF16, kind='ExternalOutput').ap()
        dbg_v = nc.dram_tensor('dbg_v', [128, NB * HPC * 128], BF16, kind='ExternalOutput').ap()
        dbg_ys = nc.dram_tensor('dbg_ys', [S, N_EMBD], BF16, kind='ExternalOutput').ap()

    def bcast_mid(t, n, width):
        # view [128, width] tile as [128, n, width] broadcasting over middle dim
        return bass.AP(tensor=t.tensor, offset=t.offset,
                       ap=[t.ap[0], [0, n], t.ap[-1]])

    def bcast_last(t, width):
        # view [128, n] tile as [128, n, width] broadcasting over last dim
        return bass.AP(tensor=t.tensor, offset=t.offset,
                       ap=[t.ap[0], t.ap[1], [0, width]])

    def view3(t, off, n, w, stride=None):
        # view [128, C] contiguous tile as [128, n, w] starting at column off
        return bass.AP(tensor=t.tensor, offset=t.offset + off,
                       ap=[t.ap[0], [w if stride is None else stride, n], [1, w]])

    with tile.TileContext(nc) as tc:
        import contextlib
        ctx = contextlib.ExitStack()
        with ctx:
            const = ctx.enter_context(tc.tile_pool(name='const', bufs=1))
            persist = ctx.enter_context(tc.tile_pool(name='persist', bufs=1))

            identb = const.tile([128, 128], BF16)
            make_identity(nc, identb)
            epst = const.tile([128, 1], F32)
            nc.vector.memset(epst, RMS_EPS)

            # persistent attention operands, bf16
            qT = persist.tile([128, NHC, S], BF16, name='qT')
            kT = persist.tile([128, NHC, S], BF16, name='kT')
            attT = persist.tile([128, NHC, S], BF16, name='attT')
            # v per head padded to 128 cols: cols 0:64 = ones, 64:128 = v, so
            # the PV matmul produces denominator rows at partitions 0:64
            # (reciprocal_approx_fast needs an offset-0 view).
            vt = [persist.tile([128, HPC, 128], BF16, name=f'vt{i}') for i in range(NB)]

            # csn[p, nb, 0:32] = cos, [32:64] = sin for position nb*128+p
            csn = const.tile([128, NB, HD], BF16, name='csn')
            nc.sync.dma_start(
                out=csn,
                in_=bass.AP(tensor=csg.tensor, offset=csg.offset,
                            ap=[[HD, 128], [128 * HD, NB], [1, HD]]))
            for nb in range(NB):
                nc.gpsimd.memset(vt[nb][:, :, 0:HD], 1.0)

            # ---- phase 1: QKV projection + rotary/RMS + head transposes ----
            with tc.tile_pool(name='xwp', bufs=1) as xwp, \
                 tc.tile_pool(name='work', bufs=2) as work, \
                 tc.tile_pool(name='psq', bufs=2, space='PSUM') as psq, \
                 tc.tile_pool(name='pst', bufs=2, space='PSUM') as pst:
                # big contiguous input DMAs first, in matmul consumption order;
                # tiny cos/sin gathers after (needed only once nb0's QKV ends)
                xT = [xwp.tile([128, S], BF16, name=f'xT{d}') for d in range(ND)]
                wq = [xwp.tile([128, JW], BF16, name=f'wq{d}') for d in range(ND)]
                for d in range(ND):
                    nc.sync.dma_start(out=xT[d], in_=xt[d * 128:(d + 1) * 128])
                    nc.sync.dma_start(out=wq[d], in_=wqkvT[d * 128:(d + 1) * 128])

                for nb in range(NB):
                    pq = psq.tile([128, 3, 512], F32, tag='pq')
                    for d in range(ND):
                        for jc in range(3):
                            nc.tensor.matmul(
                                pq[:, jc, :],
                                xT[d][:, nb * 128:(nb + 1) * 128],
                                wq[d][:, jc * 512:(jc + 1) * 512],
                                start=(d == 0), stop=(d == ND - 1))

                    # qk to bf16 sbuf; mean-square from pre-rotary values
                    # (rotation preserves norms)
                    qksb = work.tile([128, 16, 64], BF16, tag='qksb')
                    nc.scalar.copy(out=qksb, in_=view3(pq, 0, 16, 64))
                    sq = work.tile([128, 16, 64], BF16, tag='sq')
                    nc.scalar.activation(out=sq, in_=qksb, func=AF.Square)
                    ms = work.tile([128, 16], F32, tag='ms')
                    nc.vector.reduce_sum(out=ms, in_=sq, axis=mybir.AxisListType.X)
                    nc.scalar.activation(out=ms, in_=ms, func=AF.Sqrt,
                                         bias=epst, scale=1.0 / HD)
                    msb = work.tile([128, 16], BF16, tag='msb')
                    with nc.allow_low_precision(reason="bf16 rms normalizer"):
                        nc.vector.reciprocal(out=msb, in_=ms)

                    x1 = qksb[:, :, 0:32]
                    x2 = qksb[:, :, 32:64]
                    cb = bcast_mid(csn[:, nb, 0:32], 16, 32)
                    sb = bcast_mid(csn[:, nb, 32:64], 16, 32)
                    ta = work.tile([128, 16, 32], BF16, tag='ta')
                    tb = work.tile([128, 16, 32], BF16, tag='tb')
                    rot = work.tile([128, 16, 64], BF16, tag='rot')
                    nc.vector.tensor_mul(ta, x1, cb)
                    nc.vector.tensor_mul(tb, x2, sb)
                    nc.vector.tensor_add(rot[:, :, 0:32], ta, tb)
                    nc.vector.tensor_mul(ta, x2, cb)
                    nc.vector.tensor_mul(tb, x1, sb)
                    nc.vector.tensor_tensor(out=rot[:, :, 32:64], in0=ta, in1=tb,
                                            op=mybir.AluOpType.subtract)
                    qkb = work.tile([128, 16, 64], BF16, tag='qkb')
                    nc.vector.tensor_mul(qkb, rot, bcast_last(msb, 64))

                    # v: psum f32 -> bf16 sbuf (second half of the padded tile)
                    nc.scalar.copy(out=vt[nb][:, :, HD:128], in_=view3(pq, 1024, 8, 64))

                    # transpose q,k head pairs: [pos, 2hd] -> [2hd, pos]
                    tp = pst.tile([128, 8, 128], BF16, tag='tp')
                    for g in range(8):
                        nc.tensor.transpose(
                            tp[:, g, :],
                            qkb[:, 2 * g:2 * g + 2, :].rearrange("p a b -> p (a b)"),
                            identb)
                    nc.vector.tensor_copy(qT[:, :, nb * 128:(nb + 1) * 128], tp[:, 0:4, :])
                    nc.scalar.copy(out=kT[:, :, nb * 128:(nb + 1) * 128], in_=tp[:, 4:8, :])

            # ---- phase 2: attention interleaved with output projection ----
            with tc.tile_pool(name='estp', bufs=6) as estp, \
                 tc.tile_pool(name='attw', bufs=3) as attw, \
                 tc.tile_pool(name='wop', bufs=1) as wop, \
                 tc.tile_pool(name='ywork', bufs=4) as yw, \
                 tc.tile_pool(name='pssc', bufs=2, space='PSUM') as pssc, \
                 tc.tile_pool(name='pspv', bufs=1, space='PSUM') as pspv, \
                 tc.tile_pool(name='psy', bufs=2, space='PSUM') as psy:
                wo = []
                for f in range(NHC):
                    wof = wop.tile([128, N_EMBD], BF16, name=f'wo{f}')
                    nc.sync.dma_start(out=wof, in_=woT[f * 128:(f + 1) * 128])
                    wo.append(wof)

                for qg in range(2):
                    for hc in range(NHC):
                        nkc = 4 + qg * 4
                        pvt = pspv.tile([128, 2, 512], F32, tag='pv')
                        for kc in range(nkc):
                            vs = max(0, kc - qg * 4) * 128
                            diag = kc >= qg * 4
                            sct = pssc.tile([128, 2, 512], F32, tag='sc')
                            for h2 in range(2):
                                nc.tensor.matmul(
                                    sct[:, h2, vs:],
                                    kT[h2 * HD:(h2 + 1) * HD, hc, kc * 128:(kc + 1) * 128],
                                    qT[h2 * HD:(h2 + 1) * HD, hc,
                                       qg * 512 + vs:(qg + 1) * 512],
                                    start=True, stop=True,
                                    tile_position=(h2 * HD, 0))
                            est = estp.tile([128, 2, 512], BF16, tag='est')
                            # attention scale D^-0.5 folded into the exp
                            nc.scalar.activation(out=est[:, :, vs:], in_=sct[:, :, vs:],
                                                 func=AF.Exp, scale=HD ** -0.5)
                            if diag:
                                # zero above-diagonal exp(s) entries on gpsimd
                                nc.gpsimd.affine_select(
                                    out=est[:, :, vs:vs + 128],
                                    in_=est[:, :, vs:vs + 128],
                                    compare_op=mybir.AluOpType.is_ge,
                                    fill=0.0, base=0, pattern=[[0, 2], [1, 128]],
                                    channel_multiplier=-1)
                            for h2 in range(2):
                                nc.tensor.matmul(
                                    pvt[:, h2, vs:], vt[kc][:, hc * 2 + h2],
                                    est[:, h2, vs:],
                                    start=(kc == 0), stop=(kc == nkc - 1),
                                    skip_group_check=True)
                        den = attw.tile([HD, 2, 512], F32, tag='den')
                        nc.vector.reciprocal_approx_fast(den, pvt[0:HD, :, :])
                        for h2 in range(2):
                            nc.vector.tensor_mul(
                                attT[h2 * HD:(h2 + 1) * HD, hc, qg * 512:(qg + 1) * 512],
                                pvt[HD:128, h2, :], den[:, h2, :])

                    # project this half's rows while the other half's attention
                    # runs; ReduceScatter in 256-row chunks for overlap
                    for qt in range(qg * 4, qg * 4 + 4):
                        for og in range(2):
                            py = psy.tile([128, 512], F32, tag='py')
                            for f in range(NHC):
                                nc.tensor.matmul(
                                    py,
                                    attT[:, f, qt * 128:(qt + 1) * 128],
                                    wo[f][:, og * 512:(og + 1) * 512],
                                    start=(f == 0), stop=(f == NHC - 1))
                            ys = yw.tile([128, 512], BF16, tag='ys')
                            if og == 0:
                                nc.vector.tensor_copy(ys, py)
                            else:
                                nc.scalar.copy(out=ys, in_=py)
                            nc.sync.dma_start(
                                out=ypart[qt * 128:(qt + 1) * 128,
                                          og * 512:(og + 1) * 512],
                                in_=ys)
                    if DEBUG and qg == 1:
                        nc.sync.dma_start(out=dbg_q, in_=qT.rearrange("p a b -> p (a b)"))
                        nc.sync.dma_start(out=dbg_k, in_=kT.rearrange("p a b -> p (a b)"))
                        nc.sync.dma_start(out=dbg_att, in_=attT.rearrange("p a b -> p (a b)"))
                        for nb in range(NB):
                            nc.sync.dma_start(
                                out=dbg_v[:, nb * 1024:(nb + 1) * 1024],
                                in_=vt[nb].rearrange("p a b -> p (a b)"))

    nc.compile()
    return nc


def _get_nc():
    if 'nc' not in _cached:
        _cached['nc'] = _build()
    return _cached['nc']


def kernel(x, Wqkv, Wo, cos_cache, sin_cache, cu_seqlens, position_ids,
           max_seqlen, **_ignored):
    import ml_dtypes
    from concourse.bass_utils import run_bass_kernel_spmd

    BF = ml_dtypes.bfloat16
    x = np.asarray(x)
    Wqkv = np.asarray(Wqkv)
    Wo = np.asarray(Wo)
    cos_cache = np.asarray(cos_cache, dtype=np.float32)
    sin_cache = np.asarray(sin_cache, dtype=np.float32)
    position_ids = np.asarray(position_ids)

    nc = _get_nc()
    in_maps = []
    for c in range(NCORES):
        b, hh = c // 2, c % 2
        rows = slice(b * S, (b + 1) * S)
        qsl = slice(hh * HPC * HD, (hh + 1) * HPC * HD)
        ksl = slice(N_EMBD + hh * HPC * HD, N_EMBD + (hh + 1) * HPC * HD)
        vsl = slice(2 * N_EMBD + hh * HPC * HD, 2 * N_EMBD + (hh + 1) * HPC * HD)
        wqkvT_c = np.concatenate(
            [Wqkv[qsl], Wqkv[ksl], Wqkv[vsl]], axis=0).T
        woT_c = Wo[:, qsl].T
        pos = position_ids[rows]
        in_maps.append({
            'xt': np.ascontiguousarray(x[rows].T.astype(BF)),
            'wqkvT': np.ascontiguousarray(wqkvT_c.astype(BF)),
            'woT': np.ascontiguousarray(woT_c.astype(BF)),
            'csg': np.ascontiguousarray(
                np.concatenate([cos_cache[pos], sin_cache[pos]], 1).astype(BF)),
        })

    r = run_bass_kernel_spmd(nc, in_maps, list(range(NCORES)))
    out = np.empty((N, N_EMBD), dtype=np.float32)
    for b in range(B):
        # unshard the PartialSum-sharded y: add the two head-group partials
        out[b * S:(b + 1) * S] = (
            r.results[2 * b]['ypart'].astype(np.float32)
            + r.results[2 * b + 1]['ypart'].astype(np.float32))
    _cached['last_results'] = r
    return out


# revision 72
# speedup vs baseline: 1.1211x; 1.0676x over previous
"""Causal varlen self-attention (packed equal-length sequences) on 8 trn2 cores.

Sharding: 4 sequences x 2 head-groups. Core c handles sequence b = c//2 and
heads hh*8..hh*8+8 where hh = c%2. Each core computes the QKV projection for
its sequence restricted to its heads, rotary+RMSNorm, causal attention over
its 8 heads, and a partial output projection over its 512 features. The pair
of cores for a sequence ReduceScatter their partial y (each ends with half
the reduced rows); the host stitches the halves.

All matmuls run in bf16 (rel err ~4e-3, tolerance 2e-2). The host ships x
pre-transposed and weights pre-converted to bf16, so the device does no
f32 casts for weights and no x transpose. Attention computes only the
at-or-below-diagonal 128-col blocks (QK, exp, PV all sub-ranged). Softmax
uses exp without max subtraction (RMS-normed q,k bound |s| <= 8) in a
transposed scores layout [kpos, q]. Denominators come from a ones block
appended to V; normalization divides the small per-head attention output
using reciprocal_approx_fast. RMS mean-squares are computed from pre-rotary
values (rotation preserves norms), in parallel with the rotation itself.
"""
import numpy as np

N_EMBD = 1024
N_HEAD = 16
HD = 64
S = 1024
B = 4
N = B * S
NCORES = 8
HPC = 8             # heads per core
NHC = HPC // 2      # head-pair chunks per core
NB = S // 128       # row blocks per sequence
ND = N_EMBD // 128  # contraction chunks
JW = 3 * HPC * HD   # qkv feature width per core (1536)
NEG = -30000.0
RMS_EPS = 1.1920929e-07

_cached = {}
DEBUG = False


def _build():
    import concourse.bacc as bacc
    import concourse.mybir as mybir
    import concourse.tile as tile
    import concourse.bass as bass
    from concourse.masks import make_identity

    F32 = mybir.dt.float32
    BF16 = mybir.dt.bfloat16
    AF = mybir.ActivationFunctionType

    nc = bacc.Bacc('TRN2', target_bir_lowering=False, debug=False,
                   num_devices=NCORES)
    xt = nc.dram_tensor('xt', [N_EMBD, S], BF16, kind='ExternalInput').ap()
    wqkvT = nc.dram_tensor('wqkvT', [N_EMBD, JW], BF16, kind='ExternalInput').ap()
    woT = nc.dram_tensor('woT', [HPC * HD, N_EMBD], BF16, kind='ExternalInput').ap()
    # csg = [cos|sin] per position (host-gathered), one packed DMA
    csg = nc.dram_tensor('csg', [S, HD], BF16, kind='ExternalInput').ap()
    ypart = nc.dram_tensor('ypart', [S, N_EMBD], BF16, kind='ExternalOutput').ap()
    if DEBUG:
        dbg_q = nc.dram_tensor('dbg_q', [128, NHC * S], BF16, kind='ExternalOutput').ap()
        dbg_k = nc.dram_tensor('dbg_k', [128, NHC * S], BF16, kind='ExternalOutput').ap()
        dbg_att = nc.dram_tensor('dbg_att', [128, NHC * S], BF16, kind='ExternalOutput').ap()
        dbg_v = nc.dram_tensor('dbg_v', [128, NB * HPC * 128], BF16, kind='ExternalOutput').ap()
        dbg_ys = nc.dram_tensor('dbg_ys', [S, N_EMBD], BF16, kind='ExternalOutput').ap()

    def bcast_mid(t, n, width):
        # view [128, width] tile as [128, n, width] broadcasting over middle dim
        return bass.AP(tensor=t.tensor, offset=t.offset,
                       ap=[t.ap[0], [0, n], t.ap[-1]])

    def bcast_last(t, width):
        # view [128, n] tile as [128, n, width] broadcasting over last dim
        return bass.AP(tensor=t.tensor, offset=t.offset,
                       ap=[t.ap[0], t.ap[1], [0, width]])

    def view3(t, off, n, w, stride=None):
        # view [128, C] contiguous tile as [128, n, w] starting at column off
        return bass.AP(tensor=t.tensor, offset=t.offset + off,
                       ap=[t.ap[0], [w if stride is None else stride, n], [1, w]])

    with tile.TileContext(nc) as tc:
        import contextlib
        ctx = contextlib.ExitStack()
        with ctx:
            const = ctx.enter_context(tc.tile_pool(name='const', bufs=1))
            persist = ctx.enter_context(tc.tile_pool(name='persist', bufs=1))

            identb = const.tile([128, 128], BF16)
            make_identity(nc, identb)
            epst = const.tile([128, 1], F32)
            nc.vector.memset(epst, RMS_EPS)

            # persistent attention operands, bf16
            qT = persist.tile([128, NHC, S], BF16, name='qT')
            kT = persist.tile([128, NHC, S], BF16, name='kT')
            attT = persist.tile([128, NHC, S], BF16, name='attT')
            # v per head padded to 128 cols: cols 0:64 = ones, 64:128 = v, so
            # the PV matmul produces denominator rows at partitions 0:64
            # (reciprocal_approx_fast needs an offset-0 view).
            vt = [persist.tile([128, HPC, 128], BF16, name=f'vt{i}') for i in range(NB)]

            # csn[p, nb, 0:32] = cos, [32:64] = sin for position nb*128+p
            csn = const.tile([128, NB, HD], BF16, name='csn')
            nc.sync.dma_start(
                out=csn,
                in_=bass.AP(tensor=csg.tensor, offset=csg.offset,
                            ap=[[HD, 128], [128 * HD, NB], [1, HD]]))
            for nb in range(NB):
                nc.gpsimd.memset(vt[nb][:, :, 0:HD], 1.0)

            # ---- phase 1: QKV projection + rotary/RMS + head transposes ----
            with tc.tile_pool(name='xwp', bufs=1) as xwp, \
                 tc.tile_pool(name='work', bufs=2) as work, \
                 tc.tile_pool(name='psq', bufs=2, space='PSUM') as psq, \
                 tc.tile_pool(name='pst', bufs=2, space='PSUM') as pst:
                # big contiguous input DMAs first, in matmul consumption order;
                # tiny cos/sin gathers after (needed only once nb0's QKV ends)
                xT = [xwp.tile([128, S], BF16, name=f'xT{d}') for d in range(ND)]
                wq = [xwp.tile([128, JW], BF16, name=f'wq{d}') for d in range(ND)]
                for d in range(ND):
                    nc.sync.dma_start(out=xT[d], in_=xt[d * 128:(d + 1) * 128])
                    nc.sync.dma_start(out=wq[d], in_=wqkvT[d * 128:(d + 1) * 128])

                for nb in range(NB):
                    pq = psq.tile([128, 3, 512], F32, tag='pq')
                    for d in range(ND):
                        for jc in range(3):
                            nc.tensor.matmul(
                                pq[:, jc, :],
                                xT[d][:, nb * 128:(nb + 1) * 128],
                                wq[d][:, jc * 512:(jc + 1) * 512],
                                start=(d == 0), stop=(d == ND - 1))

                    # qk to bf16 sbuf; mean-square from pre-rotary values
                    # (rotation preserves norms)
                    qksb = work.tile([128, 16, 64], BF16, tag='qksb')
                    nc.scalar.copy(out=qksb, in_=view3(pq, 0, 16, 64))
                    sq = work.tile([128, 16, 64], BF16, tag='sq')
                    nc.scalar.activation(out=sq, in_=qksb, func=AF.Square)
                    ms = work.tile([128, 16], F32, tag='ms')
                    nc.vector.reduce_sum(out=ms, in_=sq, axis=mybir.AxisListType.X)
                    nc.scalar.activation(out=ms, in_=ms, func=AF.Sqrt,
                                         bias=epst, scale=1.0 / HD)
                    msb = work.tile([128, 16], BF16, tag='msb')
                    with nc.allow_low_precision(reason="bf16 rms normalizer"):
                        nc.vector.reciprocal(out=msb, in_=ms)

                    x1 = qksb[:, :, 0:32]
                    x2 = qksb[:, :, 32:64]
                    cb = bcast_mid(csn[:, nb, 0:32], 16, 32)
                    sb = bcast_mid(csn[:, nb, 32:64], 16, 32)
                    ta = work.tile([128, 16, 32], BF16, tag='ta')
                    tb = work.tile([128, 16, 32], BF16, tag='tb')
                    rot = work.tile([128, 16, 64], BF16, tag='rot')
                    nc.vector.tensor_mul(ta, x1, cb)
                    nc.vector.tensor_mul(tb, x2, sb)
                    nc.vector.tensor_add(rot[:, :, 0:32], ta, tb)
                    nc.vector.tensor_mul(ta, x2, cb)
                    nc.vector.tensor_mul(tb, x1, sb)
                    nc.vector.tensor_tensor(out=rot[:, :, 32:64], in0=ta, in1=tb,
                                            op=mybir.AluOpType.subtract)
                    qkb = work.tile([128, 16, 64], BF16, tag='qkb')
                    nc.vector.tensor_mul(qkb, rot, bcast_last(msb, 64))

                    # v: psum f32 -> bf16 sbuf (second half of the padded tile)
                    nc.scalar.copy(out=vt[nb][:, :, HD:128], in_=view3(pq, 1024, 8, 64))

                    # transpose q,k head pairs: [pos, 2hd] -> [2hd, pos]
                    tp = pst.tile([128, 8, 128], BF16, tag='tp')
                    for g in range(8):
                        nc.tensor.transpose(
                            tp[:, g, :],
                            qkb[:, 2 * g:2 * g + 2, :].rearrange("p a b -> p (a b)"),
                            identb)
                    nc.vector.tensor_copy(qT[:, :, nb * 128:(nb + 1) * 128], tp[:, 0:4, :])
                    nc.scalar.copy(out=kT[:, :, nb * 128:(nb + 1) * 128], in_=tp[:, 4:8, :])

            # ---- phase 2: attention interleaved with output projection ----
            with tc.tile_pool(name='estp', bufs=6) as estp, \
                 tc.tile_pool(name='attw', bufs=3) as attw, \
                 tc.tile_pool(name='wop', bufs=1) as wop, \
                 tc.tile_pool(name='ywork', bufs=4) as yw, \
                 tc.tile_pool(name='pssc', bufs=2, space='PSUM') as pssc, \
                 tc.tile_pool(name='pspv', bufs=1, space='PSUM') as pspv, \
                 tc.tile_pool(name='psy', bufs=2, space='PSUM') as psy:
                wo = []
                for f in range(NHC):
                    wof = wop.tile([128, N_EMBD], BF16, name=f'wo{f}')
                    nc.sync.dma_start(out=wof, in_=woT[f * 128:(f + 1) * 128])
                    wo.append(wof)

                def yproj(qt):
                    for og in range(2):
                        py = psy.tile([128, 512], F32, tag='py')
                        for f in range(NHC):
                            nc.tensor.matmul(
                                py,
                                attT[:, f, qt * 128:(qt + 1) * 128],
                                wo[f][:, og * 512:(og + 1) * 512],
                                start=(f == 0), stop=(f == NHC - 1))
                        ys = yw.tile([128, 512], BF16, tag='ys')
                        if og == 0:
                            nc.vector.tensor_copy(ys, py)
                        else:
                            nc.scalar.copy(out=ys, in_=py)
                        nc.sync.dma_start(
                            out=ypart[qt * 128:(qt + 1) * 128,
                                      og * 512:(og + 1) * 512],
                            in_=ys)

                for qg in range(2):
                    for hc in range(NHC):
                        nkc = 4 + qg * 4
                        pvt = pspv.tile([128, 2, 512], F32, tag='pv')
                        for kc in range(nkc):
                            vs = max(0, kc - qg * 4) * 128
                            diag = kc >= qg * 4
                            sct = pssc.tile([128, 2, 512], F32, tag='sc')
                            for h2 in range(2):
                                nc.tensor.matmul(
                                    sct[:, h2, vs:],
                                    kT[h2 * HD:(h2 + 1) * HD, hc, kc * 128:(kc + 1) * 128],
                                    qT[h2 * HD:(h2 + 1) * HD, hc,
                                       qg * 512 + vs:(qg + 1) * 512],
                                    start=True, stop=True,
                                    tile_position=(h2 * HD, 0))
                            est = estp.tile([128, 2, 512], BF16, tag='est')
                            # attention scale D^-0.5 folded into the exp
                            nc.scalar.activation(out=est[:, :, vs:], in_=sct[:, :, vs:],
                                                 func=AF.Exp, scale=HD ** -0.5)
                            if diag:
                                # zero above-diagonal exp(s) entries on gpsimd
                                nc.gpsimd.affine_select(
                                    out=est[:, :, vs:vs + 128],
                                    in_=est[:, :, vs:vs + 128],
                                    compare_op=mybir.AluOpType.is_ge,
                                    fill=0.0, base=0, pattern=[[0, 2], [1, 128]],
                                    channel_multiplier=-1)
                            for h2 in range(2):
                                nc.tensor.matmul(
                                    pvt[:, h2, vs:], vt[kc][:, hc * 2 + h2],
                                    est[:, h2, vs:],
                                    start=(kc == 0), stop=(kc == nkc - 1),
                                    skip_group_check=True)
                        den = attw.tile([HD, 2, 512], F32, tag='den')
                        nc.vector.reciprocal_approx_fast(den, pvt[0:HD, :, :])
                        for h2 in range(2):
                            nc.vector.tensor_mul(
                                attT[h2 * HD:(h2 + 1) * HD, hc, qg * 512:(qg + 1) * 512],
                                pvt[HD:128, h2, :], den[:, h2, :])

                        # interleave qg=0's output projection into qg=1's
                        # attention: exp-independent filler for tensor stalls
                        if qg == 1:
                            yproj(hc)
                    if qg == 1:
                        for qt in range(4, 8):
                            yproj(qt)
                    if DEBUG and qg == 1:
                        nc.sync.dma_start(out=dbg_q, in_=qT.rearrange("p a b -> p (a b)"))
                        nc.sync.dma_start(out=dbg_k, in_=kT.rearrange("p a b -> p (a b)"))
                        nc.sync.dma_start(out=dbg_att, in_=attT.rearrange("p a b -> p (a b)"))
                        for nb in range(NB):
                            nc.sync.dma_start(
                                out=dbg_v[:, nb * 1024:(nb + 1) * 1024],
                                in_=vt[nb].rearrange("p a b -> p (a b)"))

    nc.compile()
    return nc


def _get_nc():
    if 'nc' not in _cached:
        _cached['nc'] = _build()
    return _cached['nc']


def kernel(x, Wqkv, Wo, cos_cache, sin_cache, cu_seqlens, position_ids,
           max_seqlen, **_ignored):
    import ml_dtypes
    from concourse.bass_utils import run_bass_kernel_spmd

    BF = ml_dtypes.bfloat16
    x = np.asarray(x)
    Wqkv = np.asarray(Wqkv)
    Wo = np.asarray(Wo)
    cos_cache = np.asarray(cos_cache, dtype=np.float32)
    sin_cache = np.asarray(sin_cache, dtype=np.float32)
    position_ids = np.asarray(position_ids)

    nc = _get_nc()
    in_maps = []
    for c in range(NCORES):
        b, hh = c // 2, c % 2
        rows = slice(b * S, (b + 1) * S)
        qsl = slice(hh * HPC * HD, (hh + 1) * HPC * HD)
        ksl = slice(N_EMBD + hh * HPC * HD, N_EMBD + (hh + 1) * HPC * HD)
        vsl = slice(2 * N_EMBD + hh * HPC * HD, 2 * N_EMBD + (hh + 1) * HPC * HD)
        wqkvT_c = np.concatenate(
            [Wqkv[qsl], Wqkv[ksl], Wqkv[vsl]], axis=0).T
        woT_c = Wo[:, qsl].T
        pos = position_ids[rows]
        in_maps.append({
            'xt': np.ascontiguousarray(x[rows].T.astype(BF)),
            'wqkvT': np.ascontiguousarray(wqkvT_c.astype(BF)),
            'woT': np.ascontiguousarray(woT_c.astype(BF)),
            'csg': np.ascontiguousarray(
                np.concatenate([cos_cache[pos], sin_cache[pos]], 1).astype(BF)),
        })

    r = run_bass_kernel_spmd(nc, in_maps, list(range(NCORES)))
    out = np.empty((N, N_EMBD), dtype=np.float32)
    for b in range(B):
        # unshard the PartialSum-sharded y: add the two head-group partials
        out[b * S:(b + 1) * S] = (
            r.results[2 * b]['ypart'].astype(np.float32)
            + r.results[2 * b + 1]['ypart'].astype(np.float32))
    _cached['last_results'] = r
    return out
